# revision 6
# baseline (speedup 1.0000x reference)
"""Trainium2 Bass kernel: 3-layer MLP (256->256->256->128) + action masking.

Sharding: pure data parallel. The batch dim (65536) is split across 8
NeuronCores (8192 rows each); the small MLP weights are replicated.

Per-core design: DMA-roofline oriented. Per-core traffic is ~17.4 MB at
the 360 GB/s DMA model (~48.4 us), so every compute engine is budgeted
under the ~2.9 us DMA cadence of a 512-row chunk and all out-DMAs are
deferred until after every in-DMA (program order on the single SP queue
feeds the DMA device through the exclusive HWDGE chain), packing the
DMA device solid and hiding the pipeline drain behind the trailing
output stream. The batch is cut into 12 chunks of 512 rows plus a
tapered tail of 8 chunks of 256 rows: the 5-stage pipeline skew means
the last ~4 iterations happen after the final in-DMA, and small tail
chunks halve that drain latency.

Stages (one-iteration skew: A, B, C, D1, D2):
  A(c):  DMA x/mask (per-partition contiguous HBM segments; partition p
         holds rows off + nsub*p + n, the same permutation for x, mask
         and out), FLOAT_MIN fill of the out tile (GPSIMD), all-invalid
         detection (DVE reduce_max - free-axis reduces are DVE-only -
         and GPSIMD is_equal), 2*nsub PE transposes of x into
         feature-major x^T (f32r datapath, 1.5 cycles/row; f32 xbar
         DMA-transpose is unsupported), PSUM->SBUF copies split ACT/DVE.
  B(c):  L1 matmuls (f32r, moving N>=256 at 1 cycle/row) + relu/bias
         fused into the PSUM->SBUF copy (ACT).
  C(c):  L2 matmuls + relu/bias copies producing h2 in bf16
         (ACT m=0, DVE tensor_scalar m=1).
  D1(c): L3 computed *swapped*: the bf16 h2 128-column block is the
         stationary operand and bf16 W3 the moving one, so the logits
         land batch-major [128(b), nsub, 128(A)] in PSUM directly -- no
         output PE transposes and no separate bias copy. bf16 runs
         1 cycle/row at any moving size (f32r drops to 4x at N=128),
         and rounding h2/W3 to bf16 keeps end-to-end rel err ~2e-3,
         well inside the 2e-2 gate.
  D2(c): masking: copy_predicated writes logits from PSUM over the
         FLOAT_MIN-filled SBUF tile (DVE), then b3 is added to the
         whole tile as a partition-broadcast row (GPSIMD): masked
         entries stay exactly FLOAT_MIN because FLOAT_MIN + b3 rounds
         back to FLOAT_MIN in f32 (|b3| << ulp at 2^128), and the
         all-invalid col-0 := 1.0 fixup runs after the add (DVE).

Engine budget per 512-row chunk (cost model): DMA 2912 ns (bound),
PE ~2900, DVE ~2700, ACT ~2400, GPSIMD ~1700.
"""

import numpy as np

import concourse.bass as bass
import concourse.mybir as mybir
import concourse.tile as tile
from concourse import bacc
from concourse.bass_utils import run_bass_kernel_spmd
from concourse.masks import make_identity

N_CORES = 8
B, S, F1, F2, A = 65536, 256, 256, 256, 128
B_CORE = B // N_CORES   # 8192
FLOAT_MIN = float(np.finfo(np.float32).min)

# Tapered chunking: small head chunks fill the pipeline at half latency,
# small tail chunks halve the drain latency; 512-row chunks in between.
CHUNK_ROWS = [256] * 4 + [512] * 10 + [256] * 8
assert sum(CHUNK_ROWS) == B_CORE
NCHUNK = len(CHUNK_ROWS)
CHUNK_OFF = [sum(CHUNK_ROWS[:i]) for i in range(NCHUNK)]

MM_DT = mybir.dt.float32r
BF = mybir.dt.bfloat16


def _build(mm_dt=MM_DT):
    # Bacc (not plain Bass): its compile() pass splits multi-sem waits into
    # EventSemaphores — TRN2 instructions carry at most one wait, and
    # self-loading fp32/f32r matmuls can't offload waits to a LDWEIGHTS.
    nc = bacc.Bacc(None, target_bir_lowering=False)
    f32 = mybir.dt.float32
    i32 = mybir.dt.int32
    Relu = mybir.ActivationFunctionType.Relu

    obs = nc.dram_tensor("obs_state", [B_CORE, S], f32, kind="ExternalInput")[:]
    msk = nc.dram_tensor("action_mask", [B_CORE, A], i32, kind="ExternalInput")[:]
    w1 = nc.dram_tensor("W1", [S, F1], f32, kind="ExternalInput")[:]
    b1 = nc.dram_tensor("b1", [F1], f32, kind="ExternalInput")[:]
    w2 = nc.dram_tensor("W2", [F1, F2], f32, kind="ExternalInput")[:]
    b2 = nc.dram_tensor("b2", [F2], f32, kind="ExternalInput")[:]
    w3 = nc.dram_tensor("W3", [F2, A], f32, kind="ExternalInput")[:]
    b3 = nc.dram_tensor("b3", [A], f32, kind="ExternalInput")[:]
    out = nc.dram_tensor("out", [B_CORE, A], f32, kind="ExternalOutput")[:]

    def chunk_view(t, c):
        off, nb = CHUNK_OFF[c], CHUNK_ROWS[c]
        return t[off : off + nb].rearrange("(p n) f -> p n f", p=128)

    with tile.TileContext(nc) as tc:
        with (
            tc.tile_pool(name="singles", bufs=1) as singles,
            tc.tile_pool(name="stage", bufs=1) as stage,
            tc.tile_pool(name="dmat", bufs=10) as dmat,
            tc.tile_pool(name="maskp", bufs=18) as maskp,
            tc.tile_pool(name="outp", bufs=NCHUNK) as outp,
            tc.tile_pool(name="temps", bufs=3) as temps,
            tc.tile_pool(name="psum", bufs=2, space="PSUM") as psum,
        ):
            # ---- constants needed by chunk 0 (no DMA) ----
            ident = singles.tile([128, 128], f32)
            make_identity(nc, ident)
            # f32r identity: the transpose datapath runs at 1.5 cycles/row
            # for f32r (vs 2 for exact fp32).
            identr = singles.tile([128, 128], mm_dt)
            nc.scalar.copy(identr, ident)
            ones4 = singles.tile([128, 4], f32)
            nc.vector.memset(ones4, 1.0)

            mask_t, out_t, inv_t = {}, {}, {}
            xt_t, h1_t, h2_t, lp_t = {}, {}, {}, {}

            def stage_a(c):
                nb = CHUNK_ROWS[c]
                nsub = nb // 128
                # x lands in an f32r-typed tile (bitwise-identical; the
                # bitcast keeps HWDGE happy) so the PE transposes use the
                # f32r datapath. L1 consumes it as f32r anyway.
                x_sb = dmat.tile([128, nsub, S], mm_dt, tag="x", name="x")
                nc.sync.dma_start(x_sb, chunk_view(obs, c).bitcast(mm_dt))
                mask_t[c] = maskp.tile([128, nsub, A], i32, tag="mask", name="mask")
                nc.sync.dma_start(mask_t[c], chunk_view(msk, c))
                out_t[c] = outp.tile([128, nsub, A], f32, tag="out", name="outt")
                nc.gpsimd.memset(out_t[c], FLOAT_MIN)
                # All-invalid detection depends only on the mask; free-axis
                # reduces are DVE-only, the is_equal runs on GPSIMD, and the
                # tiny col-0 fixup happens in D2.
                many = temps.tile([128, nsub], i32, tag="many", name="many", bufs=8)
                nc.vector.reduce_max(
                    out=many, in_=mask_t[c], axis=mybir.AxisListType.X
                )
                inv_t[c] = temps.tile([128, nsub], i32, tag="inv", name="inv", bufs=8)
                nc.gpsimd.tensor_scalar(
                    inv_t[c], many, 0, None, mybir.AluOpType.is_equal
                )

                xt_t[c] = temps.tile([128, 2, nb], mm_dt, tag="xt", name="xt")
                for k in range(2):
                    tp = psum.tile([128, nb], mm_dt, tag="tp", bufs=2, name="tp")
                    for n in range(nsub):
                        nc.tensor.transpose(
                            tp[:, n * 128 : (n + 1) * 128],
                            x_sb[:, n, k * 128 : (k + 1) * 128],
                            identr,
                        )
                    # Split the two PSUM->SBUF copies across ACT and DVE.
                    if k == 0:
                        nc.scalar.copy(xt_t[c][:, k, :], tp)
                    else:
                        nc.vector.tensor_copy(xt_t[c][:, k, :], tp)

            def stage_b(c):
                nb = CHUNK_ROWS[c]
                xt_sb = xt_t.pop(c)
                h1_t[c] = temps.tile([128, 2, nb], mm_dt, tag="h1", name="h1")
                for m in range(2):
                    ps = psum.tile([128, nb], f32, tag="mm", bufs=4, name="mmp")
                    for k in range(2):
                        nc.tensor.matmul(
                            ps,
                            w_sb["w1"][:, k, m * 128 : (m + 1) * 128],
                            xt_sb[:, k, :],
                            start=(k == 0),
                            stop=(k == 1),
                        )
                    nc.scalar.activation(
                        h1_t[c][:, m, :], ps, Relu, bias=b1_sb[:, m : m + 1]
                    )

            def stage_c(c):
                nb = CHUNK_ROWS[c]
                h1_sb = h1_t.pop(c)
                # h2 in bf16: it is the stationary operand of the swapped L3
                # (bf16 keeps 1 cycle/row at N=128) and rounds only ~2^-9.
                h2_t[c] = temps.tile([128, 2, nb], BF, tag="h2", name="h2")
                for m in range(2):
                    ps = psum.tile([128, nb], f32, tag="mm", bufs=4, name="mmp")
                    for k in range(2):
                        nc.tensor.matmul(
                            ps,
                            w_sb["w2"][:, k, m * 128 : (m + 1) * 128],
                            h1_sb[:, k, :],
                            start=(k == 0),
                            stop=(k == 1),
                        )
                    if m == 0:
                        nc.scalar.activation(
                            h2_t[c][:, m, :], ps, Relu, bias=b2_sb[:, m : m + 1]
                        )
                    else:
                        nc.vector.tensor_scalar(
                            h2_t[c][:, m, :], ps,
                            b2_sb[:, m : m + 1], 0.0,
                            mybir.AluOpType.add, mybir.AluOpType.max,
                        )

            def stage_d1(c):
                nsub = CHUNK_ROWS[c] // 128
                # Swapped L3: stationary = h2 batch-block [f2=128, b=128],
                # moving = W3 [f2=128, A=128]; PSUM gets batch-major logits
                # [b=128, bb, A=128] with bb == the n sub-row index, matching
                # the x/mask/out partition-interleaved layout exactly.
                h2_sb = h2_t.pop(c)
                lp_t[c] = psum.tile([128, nsub, A], f32, tag="lp", bufs=2, name="lp")
                for bb in range(nsub):
                    for k in range(2):
                        nc.tensor.matmul(
                            lp_t[c][:, bb, :],
                            h2_sb[:, k, bb * 128 : (bb + 1) * 128],
                            w_sb["w3"][:, k, :],
                            start=(k == 0),
                            stop=(k == 1),
                        )

            def stage_d2(c):
                nsub = CHUNK_ROWS[c] // 128
                lp = lp_t.pop(c)
                mask_sb = mask_t.pop(c)
                out_sb = out_t[c]
                # Valid entries: logits straight from PSUM; masked entries
                # keep the FLOAT_MIN fill.
                nc.vector.copy_predicated(out_sb, mask_sb, lp)
                # b3 after predication: FLOAT_MIN + b3 rounds back to exactly
                # FLOAT_MIN (|b3| << ulp at 2^128), valid entries get +b3.
                nc.gpsimd.tensor_add(
                    out_sb, out_sb, b3_all.unsqueeze(1).broadcast_to([128, nsub, A])
                )
                # All-invalid rows: col 0 := 1.0 (after the b3 add).
                nc.vector.copy_predicated(
                    out_sb[:, :, 0], inv_t.pop(c), ones4[:, :nsub]
                )

            # Chunks 0/1's input DMAs lead the SP queue so the DMA device
            # starts on real work; the (smaller) weight loads ride behind.
            stage_a(0)
            stage_a(1)

            # ---- weights: w1/w2 land directly in f32r-typed tiles via the
            # same bitwise-identical DMA bitcast used for x (no staging copy,
            # keeps ACT/DVE free); w3's real f32->bf16 conversion runs on the
            # otherwise-idle GPSIMD.
            w_sb = {}
            for name, w, kdim, fdim in (("w1", w1, S, F1), ("w2", w2, F1, F2)):
                wr = singles.tile([128, kdim // 128, fdim], mm_dt, tag=name)
                nc.sync.dma_start(
                    wr, w.rearrange("(k p) f -> p k f", p=128).bitcast(mm_dt)
                )
                w_sb[name] = wr
            w3f = stage.tile([128, F2 // 128, A], f32, tag="st_w3")
            nc.sync.dma_start(w3f, w3.rearrange("(k p) f -> p k f", p=128))
            w3b = singles.tile([128, F2 // 128, A], BF, tag="w3")
            nc.gpsimd.tensor_copy(w3b, w3f)
            w_sb["w3"] = w3b

            b1_sb = singles.tile([128, 2], f32)
            nc.sync.dma_start(b1_sb, b1.rearrange("(k p) -> p k", p=128))
            b2_sb = singles.tile([128, 2], f32)
            nc.sync.dma_start(b2_sb, b2.rearrange("(k p) -> p k", p=128))
            # b3 indexes the action (free) dim of the batch-major logits, so
            # it is broadcast across partitions once and added as a row.
            b3_row = singles.tile([1, A], f32)
            nc.sync.dma_start(b3_row, b3.rearrange("(o a) -> o a", o=1))
            b3_all = singles.tile([128, A], f32)
            nc.gpsimd.partition_broadcast(b3_all, b3_row)

            for i in range(1, NCHUNK + 4):
                if 2 <= i + 1 < NCHUNK:
                    stage_a(i + 1)
                if 1 <= i < NCHUNK + 1:
                    stage_b(i - 1)
                if 2 <= i < NCHUNK + 2:
                    stage_c(i - 2)
                if 3 <= i < NCHUNK + 3:
                    stage_d1(i - 3)
                if 4 <= i:
                    stage_d2(i - 4)

            # Deferred output stream: every out-DMA sits after every in-DMA
            # on the SP queue, so the transfers pack back-to-back at the tail
            # instead of punching holes in the input stream.
            for c in range(NCHUNK):
                nc.sync.dma_start(chunk_view(out, c), out_t.pop(c))

    return nc


_NC_CACHE = {}


def _get_nc(mm_dt=MM_DT):
    key = str(mm_dt)
    if key not in _NC_CACHE:
        nc = _build(mm_dt)
        # Run Bacc's compile passes (wait splitting, register allocation);
        # the PJRT execute path serializes nc without finalizing it.
        nc.finalize()
        _NC_CACHE[key] = nc
    return _NC_CACHE[key]


def kernel(**inputs):
    obs = np.ascontiguousarray(np.asarray(inputs["obs_state"], dtype=np.float32))
    msk = np.ascontiguousarray(np.asarray(inputs["action_mask"], dtype=np.int32))
    weights = {
        k: np.ascontiguousarray(np.asarray(inputs[k], dtype=np.float32))
        for k in ("W1", "b1", "W2", "b2", "W3", "b3")
    }

    nc = _get_nc()
    in_maps = []
    for i in range(N_CORES):
        sl = slice(i * B_CORE, (i + 1) * B_CORE)
        in_maps.append(
            {"obs_state": obs[sl], "action_mask": msk[sl], **weights}
        )
    res = run_bass_kernel_spmd(nc, in_maps, core_ids=list(range(N_CORES)))
    return np.concatenate([r["out"] for r in res.results], axis=0)


if __name__ == "__main__":
    nc = _get_nc()
    print("build OK")


# revision 8
# speedup vs baseline: 1.0085x; 1.0085x over previous
"""Trainium2 Bass kernel: 3-layer MLP (256->256->256->128) + action masking.

Sharding: pure data parallel. The batch dim (65536) is split across 8
NeuronCores (8192 rows each); the small MLP weights are replicated.

Per-core design: DMA-roofline oriented. Per-core traffic is ~17.4 MB at
the 360 GB/s DMA model (~48.4 us), so every compute engine is budgeted
under the ~2.9 us DMA cadence of a 512-row chunk and all out-DMAs are
deferred until after every in-DMA (program order on the single SP queue
feeds the DMA device through the exclusive HWDGE chain), packing the
DMA device solid and hiding the pipeline drain behind the trailing
output stream. The batch is cut into 12 chunks of 512 rows plus a
tapered tail of 8 chunks of 256 rows: the 5-stage pipeline skew means
the last ~4 iterations happen after the final in-DMA, and small tail
chunks halve that drain latency.

Stages (one-iteration skew: A, B, C, D1, D2):
  A(c):  DMA x/mask (per-partition contiguous HBM segments; partition p
         holds rows off + nsub*p + n, the same permutation for x, mask
         and out), FLOAT_MIN fill of the out tile (GPSIMD), all-invalid
         detection (DVE reduce_max - free-axis reduces are DVE-only -
         and GPSIMD is_equal), 2*nsub PE transposes of x into
         feature-major x^T (f32r datapath, 1.5 cycles/row; f32 xbar
         DMA-transpose is unsupported), PSUM->SBUF copies split ACT/DVE.
  B(c):  L1 matmuls (f32r, moving N>=256 at 1 cycle/row) + relu/bias
         fused into the PSUM->SBUF copy (ACT).
  C(c):  L2 matmuls + relu/bias copies producing h2 in bf16
         (ACT m=0, DVE tensor_scalar m=1).
  D1(c): L3 computed *swapped*: the bf16 h2 128-column block is the
         stationary operand and bf16 W3 the moving one, so the logits
         land batch-major [128(b), nsub, 128(A)] in PSUM directly -- no
         output PE transposes and no separate bias copy. bf16 runs
         1 cycle/row at any moving size (f32r drops to 4x at N=128),
         and rounding h2/W3 to bf16 keeps end-to-end rel err ~2e-3,
         well inside the 2e-2 gate.
  D2(c): masking: copy_predicated writes logits from PSUM over the
         FLOAT_MIN-filled SBUF tile (DVE), then b3 is added to the
         whole tile as a partition-broadcast row (GPSIMD): masked
         entries stay exactly FLOAT_MIN because FLOAT_MIN + b3 rounds
         back to FLOAT_MIN in f32 (|b3| << ulp at 2^128), and the
         all-invalid col-0 := 1.0 fixup runs after the add (DVE).

Engine budget per 512-row chunk (cost model): DMA 2912 ns (bound),
PE ~2900, DVE ~2700, ACT ~2400, GPSIMD ~1700.
"""

import numpy as np

import concourse.bass as bass
import concourse.mybir as mybir
import concourse.tile as tile
from concourse import bacc
from concourse.bass_utils import run_bass_kernel_spmd
from concourse.masks import make_identity

N_CORES = 8
B, S, F1, F2, A = 65536, 256, 256, 256, 128
B_CORE = B // N_CORES   # 8192
FLOAT_MIN = float(np.finfo(np.float32).min)

# Tapered chunking: small head chunks fill the pipeline at half latency,
# small tail chunks halve the drain latency; 512-row chunks in between.
CHUNK_ROWS = [256] * 4 + [512] * 10 + [256] * 8
assert sum(CHUNK_ROWS) == B_CORE
NCHUNK = len(CHUNK_ROWS)
CHUNK_OFF = [sum(CHUNK_ROWS[:i]) for i in range(NCHUNK)]

MM_DT = mybir.dt.float32r
BF = mybir.dt.bfloat16


def _build(mm_dt=MM_DT):
    # Bacc (not plain Bass): its compile() pass splits multi-sem waits into
    # EventSemaphores — TRN2 instructions carry at most one wait, and
    # self-loading fp32/f32r matmuls can't offload waits to a LDWEIGHTS.
    nc = bacc.Bacc(None, target_bir_lowering=False)
    f32 = mybir.dt.float32
    i32 = mybir.dt.int32
    Relu = mybir.ActivationFunctionType.Relu

    obs = nc.dram_tensor("obs_state", [B_CORE, S], f32, kind="ExternalInput")[:]
    msk = nc.dram_tensor("action_mask", [B_CORE, A], i32, kind="ExternalInput")[:]
    w1 = nc.dram_tensor("W1", [S, F1], f32, kind="ExternalInput")[:]
    b1 = nc.dram_tensor("b1", [F1], f32, kind="ExternalInput")[:]
    w2 = nc.dram_tensor("W2", [F1, F2], f32, kind="ExternalInput")[:]
    b2 = nc.dram_tensor("b2", [F2], f32, kind="ExternalInput")[:]
    w3 = nc.dram_tensor("W3", [F2, A], f32, kind="ExternalInput")[:]
    b3 = nc.dram_tensor("b3", [A], f32, kind="ExternalInput")[:]
    out = nc.dram_tensor("out", [B_CORE, A], f32, kind="ExternalOutput")[:]

    def chunk_view(t, c):
        off, nb = CHUNK_OFF[c], CHUNK_ROWS[c]
        return t[off : off + nb].rearrange("(p n) f -> p n f", p=128)

    with tile.TileContext(nc) as tc:
        with (
            tc.tile_pool(name="singles", bufs=1) as singles,
            tc.tile_pool(name="stage", bufs=1) as stage,
            tc.tile_pool(name="dmat", bufs=10) as dmat,
            tc.tile_pool(name="maskp", bufs=18) as maskp,
            tc.tile_pool(name="outp", bufs=NCHUNK) as outp,
            tc.tile_pool(name="temps", bufs=3) as temps,
            tc.tile_pool(name="psum", bufs=2, space="PSUM") as psum,
        ):
            # ---- constants needed by chunk 0 (no DMA) ----
            ident = singles.tile([128, 128], f32)
            make_identity(nc, ident)
            # f32r identity: the transpose datapath runs at 1.5 cycles/row
            # for f32r (vs 2 for exact fp32).
            identr = singles.tile([128, 128], mm_dt)
            nc.scalar.copy(identr, ident)
            ones4 = singles.tile([128, 4], f32)
            nc.vector.memset(ones4, 1.0)

            mask_t, out_t, inv_t = {}, {}, {}
            xt_t, h1_t, h2_t, lp_t = {}, {}, {}, {}

            def stage_a(c):
                nb = CHUNK_ROWS[c]
                nsub = nb // 128
                # x lands in an f32r-typed tile (bitwise-identical; the
                # bitcast keeps HWDGE happy) so the PE transposes use the
                # f32r datapath. L1 consumes it as f32r anyway.
                x_sb = dmat.tile([128, nsub, S], mm_dt, tag="x", name="x")
                nc.sync.dma_start(x_sb, chunk_view(obs, c).bitcast(mm_dt))
                mask_t[c] = maskp.tile([128, nsub, A], i32, tag="mask", name="mask")
                nc.sync.dma_start(mask_t[c], chunk_view(msk, c))
                out_t[c] = outp.tile([128, nsub, A], f32, tag="out", name="outt")
                nc.gpsimd.memset(out_t[c], FLOAT_MIN)
                # All-invalid detection depends only on the mask; free-axis
                # reduces are DVE-only, the is_equal runs on GPSIMD, and the
                # tiny col-0 fixup happens in D2.
                many = temps.tile([128, nsub], i32, tag="many", name="many", bufs=8)
                nc.vector.reduce_max(
                    out=many, in_=mask_t[c], axis=mybir.AxisListType.X
                )
                inv_t[c] = temps.tile([128, nsub], i32, tag="inv", name="inv", bufs=8)
                nc.gpsimd.tensor_scalar(
                    inv_t[c], many, 0, None, mybir.AluOpType.is_equal
                )

                xt_t[c] = temps.tile([128, 2, nb], mm_dt, tag="xt", name="xt")
                for k in range(2):
                    tp = psum.tile([128, nb], mm_dt, tag="tp", bufs=2, name="tp")
                    for n in range(nsub):
                        nc.tensor.transpose(
                            tp[:, n * 128 : (n + 1) * 128],
                            x_sb[:, n, k * 128 : (k + 1) * 128],
                            identr,
                        )
                    # Split the two PSUM->SBUF copies across ACT and DVE.
                    if k == 0:
                        nc.scalar.copy(xt_t[c][:, k, :], tp)
                    else:
                        nc.vector.tensor_copy(xt_t[c][:, k, :], tp)

            def stage_b(c):
                nb = CHUNK_ROWS[c]
                xt_sb = xt_t.pop(c)
                h1_t[c] = temps.tile([128, 2, nb], mm_dt, tag="h1", name="h1")
                for m in range(2):
                    ps = psum.tile([128, nb], f32, tag="mm", bufs=4, name="mmp")
                    for k in range(2):
                        nc.tensor.matmul(
                            ps,
                            w_sb["w1"][:, k, m * 128 : (m + 1) * 128],
                            xt_sb[:, k, :],
                            start=(k == 0),
                            stop=(k == 1),
                        )
                    nc.scalar.activation(
                        h1_t[c][:, m, :], ps, Relu, bias=b1_sb[:, m : m + 1]
                    )

            def stage_c(c):
                nb = CHUNK_ROWS[c]
                h1_sb = h1_t.pop(c)
                # h2 in bf16: it is the stationary operand of the swapped L3
                # (bf16 keeps 1 cycle/row at N=128) and rounds only ~2^-9.
                h2_t[c] = temps.tile([128, 2, nb], BF, tag="h2", name="h2")
                for m in range(2):
                    ps = psum.tile([128, nb], f32, tag="mm", bufs=4, name="mmp")
                    for k in range(2):
                        nc.tensor.matmul(
                            ps,
                            w_sb["w2"][:, k, m * 128 : (m + 1) * 128],
                            h1_sb[:, k, :],
                            start=(k == 0),
                            stop=(k == 1),
                        )
                    if m == 0:
                        nc.scalar.activation(
                            h2_t[c][:, m, :], ps, Relu, bias=b2_sb[:, m : m + 1]
                        )
                    else:
                        nc.vector.tensor_scalar(
                            h2_t[c][:, m, :], ps,
                            b2_sb[:, m : m + 1], 0.0,
                            mybir.AluOpType.add, mybir.AluOpType.max,
                        )

            def stage_d1(c):
                nsub = CHUNK_ROWS[c] // 128
                # Swapped L3: stationary = h2 batch-block [f2=128, b=128],
                # moving = W3 [f2=128, A=128]; PSUM gets batch-major logits
                # [b=128, bb, A=128] with bb == the n sub-row index, matching
                # the x/mask/out partition-interleaved layout exactly.
                h2_sb = h2_t.pop(c)
                lp_t[c] = psum.tile([128, nsub, A], f32, tag="lp", bufs=2, name="lp")
                for bb in range(nsub):
                    for k in range(2):
                        nc.tensor.matmul(
                            lp_t[c][:, bb, :],
                            h2_sb[:, k, bb * 128 : (bb + 1) * 128],
                            w_sb["w3"][:, k, :],
                            start=(k == 0),
                            stop=(k == 1),
                        )

            def stage_d2(c):
                nsub = CHUNK_ROWS[c] // 128
                lp = lp_t.pop(c)
                mask_sb = mask_t.pop(c)
                out_sb = out_t[c]
                # Valid entries: logits straight from PSUM; masked entries
                # keep the FLOAT_MIN fill.
                nc.vector.copy_predicated(out_sb, mask_sb, lp)
                # b3 after predication: FLOAT_MIN + b3 rounds back to exactly
                # FLOAT_MIN (|b3| << ulp at 2^128), valid entries get +b3.
                nc.gpsimd.tensor_add(
                    out_sb, out_sb, b3_all.unsqueeze(1).broadcast_to([128, nsub, A])
                )
                # All-invalid rows: col 0 := 1.0 (after the b3 add).
                nc.vector.copy_predicated(
                    out_sb[:, :, 0], inv_t.pop(c), ones4[:, :nsub]
                )

            # Chunk input DMAs lead the SP queue so the DMA device starts on
            # real work; the (smaller) weight/bias loads interleave between
            # them so their DGE latency hides under the big transfers.
            # w1/w2 land directly in f32r-typed tiles via the same
            # bitwise-identical DMA bitcast used for x (no staging copy,
            # keeps ACT/DVE free); w3's real f32->bf16 conversion runs on the
            # otherwise-idle GPSIMD.
            w_sb = {}
            stage_a(0)
            w1r = singles.tile([128, S // 128, F1], mm_dt, tag="w1")
            nc.sync.dma_start(
                w1r, w1.rearrange("(k p) f -> p k f", p=128).bitcast(mm_dt)
            )
            w_sb["w1"] = w1r
            b1_sb = singles.tile([128, 2], f32)
            nc.sync.dma_start(b1_sb, b1.rearrange("(k p) -> p k", p=128))
            stage_a(1)
            w2r = singles.tile([128, F1 // 128, F2], mm_dt, tag="w2")
            nc.sync.dma_start(
                w2r, w2.rearrange("(k p) f -> p k f", p=128).bitcast(mm_dt)
            )
            w_sb["w2"] = w2r
            b2_sb = singles.tile([128, 2], f32)
            nc.sync.dma_start(b2_sb, b2.rearrange("(k p) -> p k", p=128))
            stage_a(2)
            w3f = stage.tile([128, F2 // 128, A], f32, tag="st_w3")
            nc.sync.dma_start(w3f, w3.rearrange("(k p) f -> p k f", p=128))
            w3b = singles.tile([128, F2 // 128, A], BF, tag="w3")
            nc.gpsimd.tensor_copy(w3b, w3f)
            w_sb["w3"] = w3b
            # b3 indexes the action (free) dim of the batch-major logits, so
            # it is broadcast across partitions once and added as a row.
            b3_row = singles.tile([1, A], f32)
            nc.sync.dma_start(b3_row, b3.rearrange("(o a) -> o a", o=1))
            b3_all = singles.tile([128, A], f32)
            nc.gpsimd.partition_broadcast(b3_all, b3_row)

            def stage_out(c):
                nc.sync.dma_start(chunk_view(out, c), out_t.pop(c))

            for i in range(1, NCHUNK + 4):
                if 3 <= i + 2 < NCHUNK:
                    stage_a(i + 2)
                if 1 <= i < NCHUNK + 1:
                    stage_b(i - 1)
                if 2 <= i < NCHUNK + 2:
                    stage_c(i - 2)
                if 3 <= i < NCHUNK + 3:
                    stage_d1(i - 3)
                if 4 <= i:
                    stage_d2(i - 4)
                # Out-DMA at +8 skew: by its turn in the SP queue the data has
                # been ready for ~4 iterations, so the queue never parks on it
                # (a same-iteration out-DMA head-of-line-blocks later in-DMAs).
                if 8 <= i:
                    stage_out(i - 8)

            for c in range(NCHUNK - 4, NCHUNK):
                stage_out(c)

    return nc


_NC_CACHE = {}


def _get_nc(mm_dt=MM_DT):
    key = str(mm_dt)
    if key not in _NC_CACHE:
        nc = _build(mm_dt)
        # Run Bacc's compile passes (wait splitting, register allocation);
        # the PJRT execute path serializes nc without finalizing it.
        nc.finalize()
        _NC_CACHE[key] = nc
    return _NC_CACHE[key]


def kernel(**inputs):
    obs = np.ascontiguousarray(np.asarray(inputs["obs_state"], dtype=np.float32))
    msk = np.ascontiguousarray(np.asarray(inputs["action_mask"], dtype=np.int32))
    weights = {
        k: np.ascontiguousarray(np.asarray(inputs[k], dtype=np.float32))
        for k in ("W1", "b1", "W2", "b2", "W3", "b3")
    }

    nc = _get_nc()
    in_maps = []
    for i in range(N_CORES):
        sl = slice(i * B_CORE, (i + 1) * B_CORE)
        in_maps.append(
            {"obs_state": obs[sl], "action_mask": msk[sl], **weights}
        )
    res = run_bass_kernel_spmd(nc, in_maps, core_ids=list(range(N_CORES)))
    return np.concatenate([r["out"] for r in res.results], axis=0)


if __name__ == "__main__":
    nc = _get_nc()
    print("build OK")
